# revision 35
# baseline (speedup 1.0000x reference)
"""Trainium2 Bass kernel for multi-head self-attention (B=2, N=2048, C=1024, H=16, d=64).

Sharding: 8 cores = 2 batches x 4 head-groups (4 heads each). Each core computes
QKV for its heads (column-sliced W_qkv), full attention over its heads, and a
row-sliced partial of the output projection. Host sums the 4 partials per batch
and adds b_proj.

Device dataflow (per core, all matmuls bf16 with fp32 PSUM accumulation):
  - x^T is loaded [C, N] so Q^T/K^T come out as [head*d, N] (d on partitions),
    which is exactly the lhsT/rhs layout the scores matmul wants.
  - S^T tile [128 keys, 512 queries] = (K^T chunk)^T-matmul(Q^T chunk), K=64
    contraction; the two heads of a pair sit at partition offsets 0/64 so their
    matmuls occupy disjoint PE row-groups and run concurrently.
  - softmax skips the max-subtraction (scores are ~N(0,1); exp is safe in fp32)
    so exp(scale*S) is a single ACT pass straight out of PSUM, cast to bf16.
  - V carries an appended ones column (65th), so the attention-output matmul
    accumulates both O^T rows (0..63) and the softmax denominators (row 64).
  - normalization: denominator row broadcast to 64 partitions via K=1
    ones-matmuls, one 128-lane reciprocal, then vector multiplies into O^T
    bf16. Its PE part is emitted two groups into the NEXT block so no PE
    instruction ever waits on a DVE copy (in-order queues).
  - projection: out[q,c] = sum_p O^T-pair-chunk^T @ W_proj rows, fp32 out via
    DMA; emission deferred into later groups to keep ACT fed.
  - scheduling: AV matmuls lag S/exp by 4 groups (software pipeline), producer
    chains drip-feed between attention groups, block order runs all pair-0
    chunks before pair-1, and the PE is HAM-warmed during the initial DMAs.
"""

import sys

sys.path.insert(0, "/opt/trn_rl_repo")

import numpy as np
import ml_dtypes

import concourse.bass as bass
import concourse.tile as tile
from concourse import bacc, mybir
from concourse.bass_utils import run_bass_kernel_spmd

BF16 = ml_dtypes.bfloat16
F32 = mybir.dt.float32
BF = mybir.dt.bfloat16
AF = mybir.ActivationFunctionType

B, NT, C, H, D = 2, 2048, 1024, 16, 64
NCORES = 8
HPC = 4  # heads per core
DQ = HPC * D  # 256 c_out per q/k/v slice
VW = HPC * (D + 1)  # 260: V with a ones column per head
SCALE = D ** -0.5


def build_program(nt=NT):
    """Build the SPMD Bass program. nt parametrized so a small version can be
    simulated quickly in CoreSim."""
    n_tc = nt // 512  # 512-token chunks
    n_kt = nt // 128  # 128-key tiles
    n_ktg = nt // 256  # groups of 2 key tiles (one exp per 1024 cols)

    nc = bacc.Bacc("TRN2", target_bir_lowering=False, debug=False,
                   num_devices=NCORES)

    xT = nc.dram_tensor("xT", [C, nt], BF, kind="ExternalInput").ap()
    wq = nc.dram_tensor("wq", [C, DQ], BF, kind="ExternalInput").ap()
    wk = nc.dram_tensor("wk", [C, DQ], BF, kind="ExternalInput").ap()
    wv = nc.dram_tensor("wv", [C, VW], BF, kind="ExternalInput").ap()
    wp = nc.dram_tensor("wp", [DQ, C], BF, kind="ExternalInput").ap()
    bqk = nc.dram_tensor("bqk", [128, 4], F32, kind="ExternalInput").ap()
    bv = nc.dram_tensor("bv", [1, VW], BF, kind="ExternalInput").ap()
    out = nc.dram_tensor("out_p", [nt, C], F32, kind="ExternalOutput").ap()

    with tile.TileContext(nc) as tc:
        with (
            tc.tile_pool(name="persist", bufs=1) as persist,
            tc.tile_pool(name="pt_pool", bufs=8) as pt_pool,
            tc.tile_pool(name="stage", bufs=4) as stage,
            tc.tile_pool(name="bcbuf", bufs=3) as bcbuf,
            tc.tile_pool(name="small", bufs=6) as small,
            tc.tile_pool(name="ps_qkv", bufs=2, space="PSUM") as ps_qkv,
            tc.tile_pool(name="ps_s", bufs=2, space="PSUM") as ps_s,
            tc.tile_pool(name="ps_o", bufs=1, space="PSUM") as ps_o,
        ):
            # ---------------- persistent SBUF state ----------------
            # load order matters: wk + xT feed the first K^T chains; wv/wp
            # are only needed once attention is underway.
            xT_sb = persist.tile([128, 8, nt], BF)
            wq_sb = persist.tile([128, 8, DQ], BF)
            wk_sb = persist.tile([128, 8, DQ], BF)
            wv_sb = persist.tile([128, 8, VW], BF)
            bqk_sb = persist.tile([128, 4], F32)
            bv_sb = persist.tile([1, VW], BF)
            wp_sb = persist.tile([128, 2, C], BF)
            # Few big DMA instructions (the ~1.3us sequencer issue cost per
            # DMA dominates; transfers run on 16 parallel DMA engines).
            # x^T rides the SP hardware queue in 512-token chunks so the first
            # K/Q chains start early; weights ride the idle Pool (SWDGE) queue.
            xT3 = xT.rearrange("(po pi) n -> pi po n", pi=128)
            wk3 = wk.rearrange("(po pi) c -> pi po c", pi=128)
            wq3 = wq.rearrange("(po pi) c -> pi po c", pi=128)
            wv3 = wv.rearrange("(po pi) c -> pi po c", pi=128)
            wp3 = wp.rearrange("(po pi) c -> pi po c", pi=128)
            def _xt(t):
                nc.sync.dma_start(xT_sb[:, :, t * 512:(t + 1) * 512],
                                  xT3[:, :, t * 512:(t + 1) * 512])

            nc.sync.dma_start(bqk_sb[:], bqk)
            nc.sync.dma_start(wk_sb[:], wk3)
            _xt(0)
            nc.sync.dma_start(wq_sb[:], wq3)
            for t in range(1, n_tc):
                _xt(t)
                if t == 1:
                    nc.sync.dma_start(bv_sb[:], bv)
                    nc.sync.dma_start(wv_sb[:], wv3)
                elif t == 2:
                    nc.sync.dma_start(wp_sb[:], wp3)
            if n_tc < 3:
                nc.sync.dma_start(bv_sb[:], bv)
                nc.sync.dma_start(wv_sb[:], wv3)
                nc.sync.dma_start(wp_sb[:], wp3)
            ones_sb = persist.tile([1, 512], BF)
            nc.vector.memset(ones_sb[:], 1.0)
            # warm the PE clock (HAM) with throwaway matmuls while the first
            # DMAs land, so the first real chains run at full rate
            warm_ps = ps_qkv.tile([128, 512], F32, tag="qkv", name="warm_ps")
            for i in range(10):
                nc.tensor.matmul(warm_ps[:, :], ones_sb[:, 0:128],
                                 ones_sb[:, :], start=(i == 0), stop=(i == 9),
                                 skip_group_check=True)
            warm_sink = persist.tile([1, 8], F32)
            nc.vector.tensor_copy(warm_sink[:, :], warm_ps[0:1, 0:8])

            qT_sb = [persist.tile([128, nt], BF, tag=f"qT{p}", name=f"qT{p}")
                     for p in range(2)]
            kT_sb = [persist.tile([128, nt], BF, tag=f"kT{p}", name=f"kT{p}")
                     for p in range(2)]
            oT_sb = [persist.tile([128, nt], BF, tag=f"oT{p}", name=f"oT{p}")
                     for p in range(2)]
            v_sb = persist.tile([128, n_kt, VW], BF)

            # ---------------- QKV chain emitters ----------------
            def qk_chain(w_sb, bcol, dst, p, t):
                ps = ps_qkv.tile([128, 512], F32, tag="qkv")
                for ci in range(8):
                    nc.tensor.matmul(
                        ps[:, :],
                        w_sb[:, ci, p * 128:(p + 1) * 128],
                        xT_sb[:, ci, t * 512:(t + 1) * 512],
                        start=(ci == 0), stop=(ci == 7))
                nc.vector.tensor_scalar_add(dst[:, t * 512:(t + 1) * 512],
                                            ps[:, :], bqk_sb[:, bcol:bcol + 1])

            def v_chain(tt):
                ps = ps_qkv.tile([128, 512], F32, tag="qkv")
                for ci in range(8):
                    nc.tensor.matmul(
                        ps[:, :VW],
                        xT_sb[:, ci, tt * 128:(tt + 1) * 128],
                        wv_sb[:, ci, :],
                        start=(ci == 0), stop=False)
                nc.tensor.matmul(
                    ps[:, :VW], ones_sb[:, 0:128], bv_sb[:, :],
                    start=False, stop=True)
                nc.vector.tensor_copy(v_sb[:, tt, :], ps[:, :VW])

            # bqk_sb columns: 0,1 = q bias pair 0/1; 2,3 = k bias pair 0/1
            def k_chain(p, t):
                qk_chain(wk_sb, 2 + p, kT_sb[p], p, t)

            def q_chain(p, t):
                qk_chain(wq_sb, 0 + p, qT_sb[p], p, t)

            # Preamble: just enough for the first attention group to start.
            k_chain(0, 0)
            q_chain(0, 0)
            for t in range(1, n_tc):
                k_chain(0, t)

            # Remaining producer chains, drip-fed between attention groups.
            # Block order runs all pair-0 query chunks first, so after the
            # preamble only the V chains (paced 2 per group by the first
            # block's own consumption) and a short K/Q chain queue remain —
            # the pair-1 chains have the whole pair-0 sweep to trickle in.
            thunks = []
            for k in range(n_ktg):
                thunks.append(lambda tt=2 * k: v_chain(tt))
                thunks.append(lambda tt=2 * k + 1: v_chain(tt))
            kq = []
            for t in range(1, n_tc):
                kq.append(lambda t=t: q_chain(0, t))
                kq.append(lambda t=t: k_chain(1, t - 1))
            kq.append(lambda: k_chain(1, n_tc - 1))
            for t in range(n_tc):
                kq.append(lambda t=t: q_chain(1, t))

            # ---------------- attention + projection ----------------
            def make_proj(qt, nh, use_act=False, pool=None):
                # use_act: at the kernel tail ACT is idle, so route the
                # PSUM->SBUF copy there and keep the DVE free for the
                # normalization multiplies
                def proj():
                    pps = (pool or ps_qkv).tile(
                        [128, 512], F32,
                        tag="s" if pool is not None else "qkv", name="pps")
                    for pp in range(2):
                        nc.tensor.matmul(
                            pps[:, :],
                            oT_sb[pp][:, qt * 128:(qt + 1) * 128],
                            wp_sb[:, pp, nh * 512:(nh + 1) * 512],
                            start=(pp == 0), stop=(pp == 1))
                    ost = stage.tile([128, 512], F32, tag="ost", name="ost")
                    if use_act:
                        nc.scalar.copy(ost[:, :], pps[:, :])
                    else:
                        nc.vector.tensor_copy(ost[:, :], pps[:, :])
                    nc.sync.dma_start(
                        out[qt * 128:(qt + 1) * 128, nh * 512:(nh + 1) * 512],
                        ost[:, :])
                return proj



            def make_norm_rest(o_ps, sumsb, p, qc, last_block):
                # PE+DVE part of softmax normalization; emitted in the NEXT
                # block once sumsb is certainly ready, so no PE instruction
                # ever waits on the DVE copy and stalls the score stream.
                def norm_rest():
                    bc_ps = ps_qkv.tile([128, 512], F32, tag="qkv",
                                        name="bc_ps")
                    for hh in range(2):
                        nc.tensor.matmul(bc_ps[hh * 64:(hh + 1) * 64, :],
                                         ones_sb[0:1, 0:64],
                                         sumsb[0:1, hh * 512:(hh + 1) * 512],
                                         start=True, stop=True,
                                         skip_group_check=True)
                    bc_sb = bcbuf.tile([128, 512], F32, tag="bc",
                                       name="bc_sb")
                    nc.vector.reciprocal(bc_sb[:, :], bc_ps[:, :])
                    for hh in range(2):
                        nc.vector.tensor_mul(
                            oT_sb[p][hh * 64:(hh + 1) * 64,
                                     qc * 512:(qc + 1) * 512],
                            o_ps[0:64, hh * 512:(hh + 1) * 512],
                            bc_sb[hh * 64:(hh + 1) * 64, :])
                    # queue this chunk's projection now that oT is written
                    if p == 1 and not last_block:
                        for qt4 in range(4):
                            for nh in range(2):
                                deferred.append(make_proj(qc * 4 + qt4, nh))
                return norm_rest

            AVLAG = min(4, n_ktg - 1)
            deferred = []
            blocks = [(qc, 0) for qc in range(n_tc)] + \
                     [(qc, 1) for qc in range(n_tc)]
            npop = 1 if n_tc >= 4 else 2
            prev_norm = None
            for bi, (qc, p) in enumerate(blocks):
                    o_ps = ps_o.tile([128, 1024], F32, tag="o", name="o_ps")
                    first_block = (bi == 0)
                    avq = []
                    for ktg in range(n_ktg):
                        if first_block:
                            for _ in range(2):
                                if thunks:
                                    thunks.pop(0)()
                        else:
                            # K/Q chains may feed this very group's S matmuls,
                            # so they must be emitted before them
                            for _ in range(npop):
                                if kq:
                                    kq.pop(0)()
                        s_ps = [ps_s.tile([128, 1024], F32, tag="s",
                                             name=f"s_ps{_h}")
                                for _h in range(2)]
                        for j in range(2):
                            kt = ktg * 2 + j
                            for hh in range(2):
                                nc.tensor.matmul(
                                    s_ps[hh][:, j * 512:(j + 1) * 512],
                                    kT_sb[p][hh * 64:(hh + 1) * 64,
                                             kt * 128:(kt + 1) * 128],
                                    qT_sb[p][hh * 64:(hh + 1) * 64,
                                             qc * 512:(qc + 1) * 512],
                                    start=True, stop=True)
                        pt = [pt_pool.tile([128, 1024], BF, tag="pt",
                                           name=f"pt{_h}")
                              for _h in range(2)]
                        for hh in range(2):
                            nc.scalar.activation(pt[hh][:, :], s_ps[hh][:, :],
                                                 AF.Exp, scale=SCALE)
                        if ktg == min(2, n_ktg - 1) and prev_norm is not None:
                            prev_norm()
                            prev_norm = None
                        else:
                            for _ in range(2 if len(deferred) > 1 else 1):
                                if deferred:
                                    deferred.pop(0)()

                        def make_av(ktg, pt):
                            def av():
                                for hh in range(2):
                                    h = 2 * p + hh
                                    for j in range(2):
                                        kt = ktg * 2 + j
                                        nc.tensor.matmul(
                                            o_ps[0:65,
                                                 hh * 512:(hh + 1) * 512],
                                            v_sb[:, kt, h * 65:(h + 1) * 65],
                                            pt[hh][:, j * 512:(j + 1) * 512],
                                            start=(ktg == 0 and j == 0),
                                            stop=(ktg == n_ktg - 1 and j == 1),
                                            skip_group_check=True)
                            return av

                        avq.append(make_av(ktg, pt))
                        if len(avq) > AVLAG:
                            avq.pop(0)()
                    while avq:
                        avq.pop(0)()
                    # start of normalization: pull the denominator row out of
                    # PSUM right after the last AV lands
                    sumsb = small.tile([1, 1024], BF, tag="sums")
                    nc.vector.tensor_copy(sumsb[:, :], o_ps[64:65, :])
                    if bi == len(blocks) - 1:
                        tail_state = (o_ps, sumsb, qc)
                        prev_norm = None
                    else:
                        prev_norm = make_norm_rest(
                            o_ps, sumsb, p, qc, last_block=False)
            # tail: last block's normalization, split per q-tile so each
            # projection starts as soon as its oT columns are normalized
            lb_o_ps, lb_sumsb, lb_qc = tail_state
            bc_ps = ps_qkv.tile([128, 512], F32, tag="qkv", name="bc_ps_t")
            for hh in range(2):
                nc.tensor.matmul(bc_ps[hh * 64:(hh + 1) * 64, :],
                                 ones_sb[0:1, 0:64],
                                 lb_sumsb[0:1, hh * 512:(hh + 1) * 512],
                                 start=True, stop=True,
                                 skip_group_check=True)
            bc_sb = bcbuf.tile([128, 512], F32, tag="bc", name="bc_sb_t")
            nc.vector.reciprocal(bc_sb[:, :], bc_ps[:, :])
            while deferred:
                deferred.pop(0)()
            for qt4 in range(4):
                qt = lb_qc * 4 + qt4
                for hh in range(2):
                    nc.vector.tensor_mul(
                        oT_sb[1][hh * 64:(hh + 1) * 64,
                                 qt * 128:(qt + 1) * 128],
                        lb_o_ps[0:64, hh * 512 + qt4 * 128:
                                hh * 512 + (qt4 + 1) * 128],
                        bc_sb[hh * 64:(hh + 1) * 64,
                              qt4 * 128:(qt4 + 1) * 128])
                for nh in range(2):
                    make_proj(qt, nh, use_act=True,
                              pool=ps_s if nh == 1 else None)()
                while deferred:
                    deferred.pop(0)()
            while deferred:
                deferred.pop(0)()
            assert not thunks and not kq, "producer chains never emitted"

    nc.finalize()
    return nc


def make_core_inputs(x, W_qkv, b_qkv, W_proj, nt=NT):
    """Host-side shard prep: returns in_maps list for the 8 cores."""
    in_maps = []
    for core in range(NCORES):
        b, g = divmod(core, NCORES // B)
        lo, hi = g * DQ, (g + 1) * DQ
        xTb = np.ascontiguousarray(x[b].T).astype(BF16)
        wq_c = np.ascontiguousarray(W_qkv[:, lo:hi]).astype(BF16)
        wk_c = np.ascontiguousarray(W_qkv[:, C + lo:C + hi]).astype(BF16)
        wv_full = W_qkv[:, 2 * C + lo:2 * C + hi]
        wv_c = np.zeros((C, VW), dtype=BF16)
        bv_c = np.zeros((1, VW), dtype=BF16)
        for h in range(HPC):
            wv_c[:, h * 65:h * 65 + 64] = wv_full[:, h * 64:(h + 1) * 64].astype(BF16)
            bv_c[0, h * 65:h * 65 + 64] = b_qkv[2 * C + lo + h * 64:
                                                2 * C + lo + (h + 1) * 64].astype(BF16)
            bv_c[0, h * 65 + 64] = 1.0
        wp_c = np.ascontiguousarray(W_proj[lo:hi, :]).astype(BF16)
        bqk_c = np.stack([
            b_qkv[lo:lo + 128], b_qkv[lo + 128:hi],
            b_qkv[C + lo:C + lo + 128], b_qkv[C + lo + 128:C + hi],
        ], axis=1).astype(np.float32)
        in_maps.append({
            "xT": xTb[:, :nt].copy(), "wq": wq_c, "wk": wk_c, "wv": wv_c,
            "wp": wp_c, "bqk": bqk_c, "bv": bv_c,
        })
    return in_maps


_prog_cache = {}


def _get_program(nt=NT):
    if nt not in _prog_cache:
        _prog_cache[nt] = build_program(nt)
    return _prog_cache[nt]


def kernel(x, W_qkv, b_qkv, W_proj, b_proj, _run_kwargs=None):
    x = np.asarray(x, dtype=np.float32)
    W_qkv = np.asarray(W_qkv, dtype=np.float32)
    b_qkv = np.asarray(b_qkv, dtype=np.float32)
    W_proj = np.asarray(W_proj, dtype=np.float32)
    b_proj = np.asarray(b_proj, dtype=np.float32)

    nc = _get_program()
    in_maps = make_core_inputs(x, W_qkv, b_qkv, W_proj)
    res = run_bass_kernel_spmd(nc, in_maps, core_ids=list(range(NCORES)),
                               **(_run_kwargs or {}))
    out = np.zeros((B, NT, C), dtype=np.float32)
    for core in range(NCORES):
        b = core // (NCORES // B)
        out[b] += res.results[core]["out_p"]
    out += b_proj[None, None, :]
    if _run_kwargs:
        kernel.last_results = res
    return out


# revision 39
# speedup vs baseline: 1.0018x; 1.0018x over previous
"""Trainium2 Bass kernel for multi-head self-attention (B=2, N=2048, C=1024, H=16, d=64).

Sharding: 8 cores = 2 batches x 4 head-groups (4 heads each). Each core computes
QKV for its heads (column-sliced W_qkv), full attention over its heads, and a
row-sliced partial of the output projection. Host sums the 4 partials per batch
and adds b_proj.

Device dataflow (per core, all matmuls bf16 with fp32 PSUM accumulation):
  - x^T is loaded [C, N] so Q^T/K^T come out as [head*d, N] (d on partitions),
    which is exactly the lhsT/rhs layout the scores matmul wants.
  - S^T tile [128 keys, 512 queries] = (K^T chunk)^T-matmul(Q^T chunk), K=64
    contraction; the two heads of a pair sit at partition offsets 0/64 so their
    matmuls occupy disjoint PE row-groups and run concurrently.
  - softmax skips the max-subtraction (scores are ~N(0,1); exp is safe in fp32)
    so exp(scale*S) is a single ACT pass straight out of PSUM, cast to bf16.
  - V carries an appended ones column (65th), so the attention-output matmul
    accumulates both O^T rows (0..63) and the softmax denominators (row 64).
  - normalization: denominator row broadcast to 64 partitions via K=1
    ones-matmuls, one 128-lane reciprocal, then vector multiplies into O^T
    bf16. Its PE part is emitted two groups into the NEXT block so no PE
    instruction ever waits on a DVE copy (in-order queues).
  - projection: out[q,c] = sum_p O^T-pair-chunk^T @ W_proj rows, fp32 out via
    DMA; emission deferred into later groups to keep ACT fed.
  - scheduling: AV matmuls lag S/exp by 4 groups (software pipeline), producer
    chains drip-feed between attention groups, block order runs all pair-0
    chunks before pair-1, and the PE is HAM-warmed during the initial DMAs.
"""

import sys

sys.path.insert(0, "/opt/trn_rl_repo")

import numpy as np
import ml_dtypes

import concourse.bass as bass
import concourse.tile as tile
from concourse import bacc, mybir
from concourse.bass_utils import run_bass_kernel_spmd

BF16 = ml_dtypes.bfloat16
F32 = mybir.dt.float32
BF = mybir.dt.bfloat16
AF = mybir.ActivationFunctionType

B, NT, C, H, D = 2, 2048, 1024, 16, 64
NCORES = 8
HPC = 4  # heads per core
DQ = HPC * D  # 256 c_out per q/k/v slice
VW = HPC * (D + 1)  # 260: V with a ones column per head
SCALE = D ** -0.5


def build_program(nt=NT):
    """Build the SPMD Bass program. nt parametrized so a small version can be
    simulated quickly in CoreSim."""
    n_tc = nt // 512  # 512-token chunks
    n_kt = nt // 128  # 128-key tiles
    n_ktg = nt // 256  # groups of 2 key tiles (one exp per 1024 cols)

    nc = bacc.Bacc("TRN2", target_bir_lowering=False, debug=False,
                   num_devices=NCORES)

    xT = nc.dram_tensor("xT", [C, nt], BF, kind="ExternalInput").ap()
    wq = nc.dram_tensor("wq", [C, DQ], BF, kind="ExternalInput").ap()
    wk = nc.dram_tensor("wk", [C, DQ], BF, kind="ExternalInput").ap()
    wv = nc.dram_tensor("wv", [C, VW], BF, kind="ExternalInput").ap()
    wp = nc.dram_tensor("wp", [DQ, C], BF, kind="ExternalInput").ap()
    bqk = nc.dram_tensor("bqk", [128, 4], F32, kind="ExternalInput").ap()
    bv = nc.dram_tensor("bv", [1, VW], BF, kind="ExternalInput").ap()
    out = nc.dram_tensor("out_p", [nt, C], F32, kind="ExternalOutput").ap()

    with tile.TileContext(nc) as tc:
        with (
            tc.tile_pool(name="persist", bufs=1) as persist,
            tc.tile_pool(name="pt_pool", bufs=8) as pt_pool,
            tc.tile_pool(name="stage", bufs=4) as stage,
            tc.tile_pool(name="bcbuf", bufs=3) as bcbuf,
            tc.tile_pool(name="small", bufs=6) as small,
            tc.tile_pool(name="ps_qkv", bufs=2, space="PSUM") as ps_qkv,
            tc.tile_pool(name="ps_s", bufs=2, space="PSUM") as ps_s,
            tc.tile_pool(name="ps_o", bufs=1, space="PSUM") as ps_o,
        ):
            # ---------------- persistent SBUF state ----------------
            # load order matters: wk + xT feed the first K^T chains; wv/wp
            # are only needed once attention is underway.
            xT_sb = persist.tile([128, 8, nt], BF)
            wq_sb = persist.tile([128, 8, DQ], BF)
            wk_sb = persist.tile([128, 8, DQ], BF)
            wv_sb = persist.tile([128, 8, VW], BF)
            bqk_sb = persist.tile([128, 4], F32)
            bv_sb = persist.tile([1, VW], BF)
            wp_sb = persist.tile([128, 2, C], BF)
            # Few big DMA instructions (the ~1.3us sequencer issue cost per
            # DMA dominates; transfers run on 16 parallel DMA engines).
            # x^T rides the SP hardware queue in 512-token chunks so the first
            # K/Q chains start early; weights ride the idle Pool (SWDGE) queue.
            xT3 = xT.rearrange("(po pi) n -> pi po n", pi=128)
            wk3 = wk.rearrange("(po pi) c -> pi po c", pi=128)
            wq3 = wq.rearrange("(po pi) c -> pi po c", pi=128)
            wv3 = wv.rearrange("(po pi) c -> pi po c", pi=128)
            wp3 = wp.rearrange("(po pi) c -> pi po c", pi=128)
            def _xt(t):
                nc.sync.dma_start(xT_sb[:, :, t * 512:(t + 1) * 512],
                                  xT3[:, :, t * 512:(t + 1) * 512])

            nc.sync.dma_start(bqk_sb[:], bqk)
            nc.sync.dma_start(wk_sb[:], wk3)
            _xt(0)
            nc.sync.dma_start(wq_sb[:], wq3)
            for t in range(1, n_tc):
                _xt(t)
                if t == 1:
                    nc.sync.dma_start(bv_sb[:], bv)
                    nc.sync.dma_start(wv_sb[:], wv3)
                elif t == 2:
                    nc.sync.dma_start(wp_sb[:], wp3)
            if n_tc < 3:
                nc.sync.dma_start(bv_sb[:], bv)
                nc.sync.dma_start(wv_sb[:], wv3)
                nc.sync.dma_start(wp_sb[:], wp3)
            ones_sb = persist.tile([1, 512], BF)
            nc.vector.memset(ones_sb[:], 1.0)
            # warm the PE clock (HAM) with throwaway matmuls while the first
            # DMAs land, so the first real chains run at full rate
            warm_ps = ps_qkv.tile([128, 512], F32, tag="qkv", name="warm_ps")
            for i in range(10):
                nc.tensor.matmul(warm_ps[:, :], ones_sb[:, 0:128],
                                 ones_sb[:, :], start=(i == 0), stop=(i == 9),
                                 skip_group_check=True)
            warm_sink = persist.tile([1, 8], F32)
            nc.vector.tensor_copy(warm_sink[:, :], warm_ps[0:1, 0:8])

            qT_sb = [persist.tile([128, nt], BF, tag=f"qT{p}", name=f"qT{p}")
                     for p in range(2)]
            kT_sb = [persist.tile([128, nt], BF, tag=f"kT{p}", name=f"kT{p}")
                     for p in range(2)]
            oT_sb = [persist.tile([128, nt], BF, tag=f"oT{p}", name=f"oT{p}")
                     for p in range(2)]
            v_sb = persist.tile([128, n_kt, VW], BF)

            # ---------------- QKV chain emitters ----------------
            def qk_chain(w_sb, bcol, dst, p, t):
                ps = ps_qkv.tile([128, 512], F32, tag="qkv")
                for ci in range(8):
                    nc.tensor.matmul(
                        ps[:, :],
                        w_sb[:, ci, p * 128:(p + 1) * 128],
                        xT_sb[:, ci, t * 512:(t + 1) * 512],
                        start=(ci == 0), stop=(ci == 7))
                nc.vector.tensor_scalar_add(dst[:, t * 512:(t + 1) * 512],
                                            ps[:, :], bqk_sb[:, bcol:bcol + 1])

            def v_chain(tt):
                ps = ps_qkv.tile([128, 512], F32, tag="qkv")
                for ci in range(8):
                    nc.tensor.matmul(
                        ps[:, :VW],
                        xT_sb[:, ci, tt * 128:(tt + 1) * 128],
                        wv_sb[:, ci, :],
                        start=(ci == 0), stop=False)
                nc.tensor.matmul(
                    ps[:, :VW], ones_sb[:, 0:128], bv_sb[:, :],
                    start=False, stop=True)
                nc.vector.tensor_copy(v_sb[:, tt, :], ps[:, :VW])

            # bqk_sb columns: 0,1 = q bias pair 0/1; 2,3 = k bias pair 0/1
            def k_chain(p, t):
                qk_chain(wk_sb, 2 + p, kT_sb[p], p, t)

            def q_chain(p, t):
                qk_chain(wq_sb, 0 + p, qT_sb[p], p, t)

            # Preamble: just enough for the first attention group to start.
            k_chain(0, 0)
            q_chain(0, 0)
            for t in range(1, n_tc):
                k_chain(0, t)

            # Remaining producer chains, drip-fed between attention groups.
            # Block order runs all pair-0 query chunks first, so after the
            # preamble only the V chains (paced 2 per group by the first
            # block's own consumption) and a short K/Q chain queue remain —
            # the pair-1 chains have the whole pair-0 sweep to trickle in.
            thunks = []
            for k in range(n_ktg):
                thunks.append(lambda tt=2 * k: v_chain(tt))
                thunks.append(lambda tt=2 * k + 1: v_chain(tt))
            kq = []
            for t in range(1, n_tc):
                kq.append(lambda t=t: q_chain(0, t))
                kq.append(lambda t=t: k_chain(1, t - 1))
            kq.append(lambda: k_chain(1, n_tc - 1))
            for t in range(n_tc):
                kq.append(lambda t=t: q_chain(1, t))

            # ---------------- attention + projection ----------------
            def make_proj(qt, nh, use_act=False, pool=None):
                # use_act: at the kernel tail ACT is idle, so route the
                # PSUM->SBUF copy there and keep the DVE free for the
                # normalization multiplies
                def proj():
                    pps = (pool or ps_qkv).tile(
                        [128, 512], F32,
                        tag="s" if pool is not None else "qkv", name="pps")
                    for pp in range(2):
                        nc.tensor.matmul(
                            pps[:, :],
                            oT_sb[pp][:, qt * 128:(qt + 1) * 128],
                            wp_sb[:, pp, nh * 512:(nh + 1) * 512],
                            start=(pp == 0), stop=(pp == 1))
                    ost = stage.tile([128, 512], F32, tag="ost", name="ost")
                    if use_act:
                        nc.scalar.copy(ost[:, :], pps[:, :])
                    else:
                        nc.vector.tensor_copy(ost[:, :], pps[:, :])
                    nc.sync.dma_start(
                        out[qt * 128:(qt + 1) * 128, nh * 512:(nh + 1) * 512],
                        ost[:, :])
                return proj



            def make_norm_rest(o_ps, sumsb, p, qc, last_block):
                # PE+DVE part of softmax normalization; emitted in the NEXT
                # block once sumsb is certainly ready, so no PE instruction
                # ever waits on the DVE copy and stalls the score stream.
                def norm_rest():
                    bc_ps = ps_qkv.tile([128, 512], F32, tag="qkv",
                                        name="bc_ps")
                    for hh in range(2):
                        nc.tensor.matmul(bc_ps[hh * 64:(hh + 1) * 64, :],
                                         ones_sb[0:1, 0:64],
                                         sumsb[0:1, hh * 512:(hh + 1) * 512],
                                         start=True, stop=True,
                                         skip_group_check=True)
                    bc_sb = bcbuf.tile([128, 512], F32, tag="bc",
                                       name="bc_sb")
                    nc.vector.reciprocal(bc_sb[:, :], bc_ps[:, :])
                    for hh in range(2):
                        nc.vector.tensor_mul(
                            oT_sb[p][hh * 64:(hh + 1) * 64,
                                     qc * 512:(qc + 1) * 512],
                            o_ps[0:64, hh * 512:(hh + 1) * 512],
                            bc_sb[hh * 64:(hh + 1) * 64, :])
                    # queue this chunk's projection now that oT is written
                    if p == 1 and not last_block:
                        for qt4 in range(4):
                            for nh in range(2):
                                deferred.append(make_proj(qc * 4 + qt4, nh))
                return norm_rest

            AVLAG = min(3, n_ktg - 1)
            deferred = []
            blocks = [(qc, 0) for qc in range(n_tc)] + \
                     [(qc, 1) for qc in range(n_tc)]
            npop = 1 if n_tc >= 4 else 2
            prev_norm = None
            for bi, (qc, p) in enumerate(blocks):
                    o_ps = ps_o.tile([128, 1024], F32, tag="o", name="o_ps")
                    first_block = (bi == 0)
                    avq = []
                    for ktg in range(n_ktg):
                        if first_block:
                            for _ in range(2):
                                if thunks:
                                    thunks.pop(0)()
                        else:
                            # K/Q chains may feed this very group's S matmuls,
                            # so they must be emitted before them
                            for _ in range(npop):
                                if kq:
                                    kq.pop(0)()
                        s_ps = [ps_s.tile([128, 1024], F32, tag="s",
                                             name=f"s_ps{_h}")
                                for _h in range(2)]
                        for j in range(2):
                            kt = ktg * 2 + j
                            for hh in range(2):
                                nc.tensor.matmul(
                                    s_ps[hh][:, j * 512:(j + 1) * 512],
                                    kT_sb[p][hh * 64:(hh + 1) * 64,
                                             kt * 128:(kt + 1) * 128],
                                    qT_sb[p][hh * 64:(hh + 1) * 64,
                                             qc * 512:(qc + 1) * 512],
                                    start=True, stop=True)
                        pt = [pt_pool.tile([128, 1024], BF, tag="pt",
                                           name=f"pt{_h}")
                              for _h in range(2)]
                        for hh in range(2):
                            nc.scalar.activation(pt[hh][:, :], s_ps[hh][:, :],
                                                 AF.Exp, scale=SCALE)
                        if ktg == min(2, n_ktg - 1) and prev_norm is not None:
                            prev_norm()
                            prev_norm = None
                        else:
                            for _ in range(2 if len(deferred) > 1 else 1):
                                if deferred:
                                    deferred.pop(0)()

                        def make_av(ktg, pt):
                            def av():
                                for hh in range(2):
                                    h = 2 * p + hh
                                    for j in range(2):
                                        kt = ktg * 2 + j
                                        nc.tensor.matmul(
                                            o_ps[0:65,
                                                 hh * 512:(hh + 1) * 512],
                                            v_sb[:, kt, h * 65:(h + 1) * 65],
                                            pt[hh][:, j * 512:(j + 1) * 512],
                                            start=(ktg == 0 and j == 0),
                                            stop=(ktg == n_ktg - 1 and j == 1),
                                            skip_group_check=True)
                            return av

                        avq.append(make_av(ktg, pt))
                        if len(avq) > AVLAG:
                            avq.pop(0)()
                    while avq:
                        avq.pop(0)()
                    # start of normalization: pull the denominator row out of
                    # PSUM right after the last AV lands
                    sumsb = small.tile([1, 1024], BF, tag="sums")
                    nc.vector.tensor_copy(sumsb[:, :], o_ps[64:65, :])
                    if bi == len(blocks) - 1:
                        tail_state = (o_ps, sumsb, qc)
                        prev_norm = None
                    else:
                        prev_norm = make_norm_rest(
                            o_ps, sumsb, p, qc, last_block=False)
            # tail: last block's normalization, split per q-tile so each
            # projection starts as soon as its oT columns are normalized
            lb_o_ps, lb_sumsb, lb_qc = tail_state
            bc_ps = ps_qkv.tile([128, 512], F32, tag="qkv", name="bc_ps_t")
            for hh in range(2):
                nc.tensor.matmul(bc_ps[hh * 64:(hh + 1) * 64, :],
                                 ones_sb[0:1, 0:64],
                                 lb_sumsb[0:1, hh * 512:(hh + 1) * 512],
                                 start=True, stop=True,
                                 skip_group_check=True)
            bc_sb = bcbuf.tile([128, 512], F32, tag="bc", name="bc_sb_t")
            nc.vector.reciprocal(bc_sb[:, :], bc_ps[:, :])
            while deferred:
                deferred.pop(0)()
            for qt4 in range(4):
                qt = lb_qc * 4 + qt4
                for hh in range(2):
                    nc.vector.tensor_mul(
                        oT_sb[1][hh * 64:(hh + 1) * 64,
                                 qt * 128:(qt + 1) * 128],
                        lb_o_ps[0:64, hh * 512 + qt4 * 128:
                                hh * 512 + (qt4 + 1) * 128],
                        bc_sb[hh * 64:(hh + 1) * 64,
                              qt4 * 128:(qt4 + 1) * 128])
                for nh in range(2):
                    make_proj(qt, nh, use_act=True,
                              pool=ps_s if nh == 1 else None)()
                while deferred:
                    deferred.pop(0)()
            while deferred:
                deferred.pop(0)()
            assert not thunks and not kq, "producer chains never emitted"

    nc.finalize()
    return nc


def make_core_inputs(x, W_qkv, b_qkv, W_proj, nt=NT):
    """Host-side shard prep: returns in_maps list for the 8 cores."""
    in_maps = []
    for core in range(NCORES):
        b, g = divmod(core, NCORES // B)
        lo, hi = g * DQ, (g + 1) * DQ
        xTb = np.ascontiguousarray(x[b].T).astype(BF16)
        wq_c = np.ascontiguousarray(W_qkv[:, lo:hi]).astype(BF16)
        wk_c = np.ascontiguousarray(W_qkv[:, C + lo:C + hi]).astype(BF16)
        wv_full = W_qkv[:, 2 * C + lo:2 * C + hi]
        wv_c = np.zeros((C, VW), dtype=BF16)
        bv_c = np.zeros((1, VW), dtype=BF16)
        for h in range(HPC):
            wv_c[:, h * 65:h * 65 + 64] = wv_full[:, h * 64:(h + 1) * 64].astype(BF16)
            bv_c[0, h * 65:h * 65 + 64] = b_qkv[2 * C + lo + h * 64:
                                                2 * C + lo + (h + 1) * 64].astype(BF16)
            bv_c[0, h * 65 + 64] = 1.0
        wp_c = np.ascontiguousarray(W_proj[lo:hi, :]).astype(BF16)
        bqk_c = np.stack([
            b_qkv[lo:lo + 128], b_qkv[lo + 128:hi],
            b_qkv[C + lo:C + lo + 128], b_qkv[C + lo + 128:C + hi],
        ], axis=1).astype(np.float32)
        in_maps.append({
            "xT": xTb[:, :nt].copy(), "wq": wq_c, "wk": wk_c, "wv": wv_c,
            "wp": wp_c, "bqk": bqk_c, "bv": bv_c,
        })
    return in_maps


_prog_cache = {}


def _get_program(nt=NT):
    if nt not in _prog_cache:
        _prog_cache[nt] = build_program(nt)
    return _prog_cache[nt]


def kernel(x, W_qkv, b_qkv, W_proj, b_proj, _run_kwargs=None):
    x = np.asarray(x, dtype=np.float32)
    W_qkv = np.asarray(W_qkv, dtype=np.float32)
    b_qkv = np.asarray(b_qkv, dtype=np.float32)
    W_proj = np.asarray(W_proj, dtype=np.float32)
    b_proj = np.asarray(b_proj, dtype=np.float32)

    nc = _get_program()
    in_maps = make_core_inputs(x, W_qkv, b_qkv, W_proj)
    res = run_bass_kernel_spmd(nc, in_maps, core_ids=list(range(NCORES)),
                               **(_run_kwargs or {}))
    out = np.zeros((B, NT, C), dtype=np.float32)
    for core in range(NCORES):
        b = core // (NCORES // B)
        out[b] += res.results[core]["out_p"]
    out += b_proj[None, None, :]
    if _run_kwargs:
        kernel.last_results = res
    return out


# revision 43
# speedup vs baseline: 1.0049x; 1.0031x over previous
"""Trainium2 Bass kernel for multi-head self-attention (B=2, N=2048, C=1024, H=16, d=64).

Sharding: 8 cores = 2 batches x 4 head-groups (4 heads each). Each core computes
QKV for its heads (column-sliced W_qkv), full attention over its heads, and a
row-sliced partial of the output projection. Host sums the 4 partials per batch
and adds b_proj.

Device dataflow (per core, all matmuls bf16 with fp32 PSUM accumulation):
  - x^T is loaded [C, N] so Q^T/K^T come out as [head*d, N] (d on partitions),
    which is exactly the lhsT/rhs layout the scores matmul wants.
  - S^T tile [128 keys, 512 queries] = (K^T chunk)^T-matmul(Q^T chunk), K=64
    contraction; the two heads of a pair sit at partition offsets 0/64 so their
    matmuls occupy disjoint PE row-groups and run concurrently.
  - softmax skips the max-subtraction (scores are ~N(0,1); exp is safe in fp32)
    so exp(scale*S) is a single ACT pass straight out of PSUM, cast to bf16.
  - V carries an appended ones column (65th), so the attention-output matmul
    accumulates both O^T rows (0..63) and the softmax denominators (row 64).
  - normalization: denominator row broadcast to 64 partitions via K=1
    ones-matmuls, one 128-lane reciprocal, then vector multiplies into O^T
    bf16. Its PE part is emitted two groups into the NEXT block so no PE
    instruction ever waits on a DVE copy (in-order queues).
  - projection: out[q,c] = sum_p O^T-pair-chunk^T @ W_proj rows, fp32 out via
    DMA; emission deferred into later groups to keep ACT fed.
  - scheduling: AV matmuls lag S/exp by 4 groups (software pipeline), producer
    chains drip-feed between attention groups, block order runs all pair-0
    chunks before pair-1, and the PE is HAM-warmed during the initial DMAs.
"""

import sys

sys.path.insert(0, "/opt/trn_rl_repo")

import numpy as np
import ml_dtypes

import concourse.bass as bass
import concourse.tile as tile
from concourse import bacc, mybir
from concourse.bass_utils import run_bass_kernel_spmd

BF16 = ml_dtypes.bfloat16
F32 = mybir.dt.float32
BF = mybir.dt.bfloat16
AF = mybir.ActivationFunctionType

B, NT, C, H, D = 2, 2048, 1024, 16, 64
NCORES = 8
HPC = 4  # heads per core
DQ = HPC * D  # 256 c_out per q/k/v slice
VW = HPC * (D + 1)  # 260: V with a ones column per head
SCALE = D ** -0.5


def build_program(nt=NT):
    """Build the SPMD Bass program. nt parametrized so a small version can be
    simulated quickly in CoreSim."""
    n_tc = nt // 512  # 512-token chunks
    n_kt = nt // 128  # 128-key tiles
    n_ktg = nt // 256  # groups of 2 key tiles (one exp per 1024 cols)

    nc = bacc.Bacc("TRN2", target_bir_lowering=False, debug=False,
                   num_devices=NCORES)

    xT = nc.dram_tensor("xT", [C, nt], BF, kind="ExternalInput").ap()
    wq = nc.dram_tensor("wq", [C, DQ], BF, kind="ExternalInput").ap()
    wk = nc.dram_tensor("wk", [C, DQ], BF, kind="ExternalInput").ap()
    wv = nc.dram_tensor("wv", [C, VW], BF, kind="ExternalInput").ap()
    wp = nc.dram_tensor("wp", [DQ, C], BF, kind="ExternalInput").ap()
    bqk = nc.dram_tensor("bqk", [128, 4], F32, kind="ExternalInput").ap()
    bv = nc.dram_tensor("bv", [1, VW], BF, kind="ExternalInput").ap()
    out = nc.dram_tensor("out_p", [nt, C], F32, kind="ExternalOutput").ap()

    with tile.TileContext(nc) as tc:
        with (
            tc.tile_pool(name="persist", bufs=1) as persist,
            tc.tile_pool(name="pt_pool", bufs=8) as pt_pool,
            tc.tile_pool(name="stage", bufs=4) as stage,
            tc.tile_pool(name="bcbuf", bufs=3) as bcbuf,
            tc.tile_pool(name="small", bufs=6) as small,
            tc.tile_pool(name="ps_qkv", bufs=2, space="PSUM") as ps_qkv,
            tc.tile_pool(name="ps_s", bufs=2, space="PSUM") as ps_s,
            tc.tile_pool(name="ps_o", bufs=1, space="PSUM") as ps_o,
        ):
            # ---------------- persistent SBUF state ----------------
            # load order matters: wk + xT feed the first K^T chains; wv/wp
            # are only needed once attention is underway.
            xT_sb = persist.tile([128, 8, nt], BF)
            wq_sb = persist.tile([128, 8, DQ], BF)
            wk_sb = persist.tile([128, 8, DQ], BF)
            wv_sb = persist.tile([128, 8, VW], BF)
            bqk_sb = persist.tile([128, 4], F32)
            bv_sb = persist.tile([1, VW], BF)
            wp_sb = persist.tile([128, 2, C], BF)
            # Few big DMA instructions (the ~1.3us sequencer issue cost per
            # DMA dominates; transfers run on 16 parallel DMA engines).
            # x^T rides the SP hardware queue in 512-token chunks so the first
            # K/Q chains start early; weights ride the idle Pool (SWDGE) queue.
            xT3 = xT.rearrange("(po pi) n -> pi po n", pi=128)
            wk3 = wk.rearrange("(po pi) c -> pi po c", pi=128)
            wq3 = wq.rearrange("(po pi) c -> pi po c", pi=128)
            wv3 = wv.rearrange("(po pi) c -> pi po c", pi=128)
            wp3 = wp.rearrange("(po pi) c -> pi po c", pi=128)
            def _xt(t):
                if t == 0:
                    return
                nc.sync.dma_start(xT_sb[:, :, t * 512:(t + 1) * 512],
                                  xT3[:, :, t * 512:(t + 1) * 512])

            nc.sync.dma_start(bqk_sb[:], bqk)
            nc.sync.dma_start(wk_sb[:], wk3)
            # first token chunk in two halves so the first K-chain's matmuls
            # start as soon as contraction-chunks 0..3 land
            nc.sync.dma_start(xT_sb[:, 0:4, 0:512], xT3[:, 0:4, 0:512])
            nc.sync.dma_start(xT_sb[:, 4:8, 0:512], xT3[:, 4:8, 0:512])
            nc.sync.dma_start(wq_sb[:], wq3)
            for t in range(1, n_tc):
                _xt(t)
                if t == 1:
                    nc.sync.dma_start(bv_sb[:], bv)
                    nc.sync.dma_start(wv_sb[:], wv3)
                elif t == 2:
                    nc.sync.dma_start(wp_sb[:], wp3)
            if n_tc < 3:
                nc.sync.dma_start(bv_sb[:], bv)
                nc.sync.dma_start(wv_sb[:], wv3)
                nc.sync.dma_start(wp_sb[:], wp3)
            ones_sb = persist.tile([1, 512], BF)
            nc.vector.memset(ones_sb[:], 1.0)
            # warm the PE clock (HAM) with throwaway matmuls while the first
            # DMAs land, so the first real chains run at full rate
            warm_ps = ps_qkv.tile([128, 512], F32, tag="qkv", name="warm_ps")
            for i in range(10):
                nc.tensor.matmul(warm_ps[:, :], ones_sb[:, 0:128],
                                 ones_sb[:, :], start=(i == 0), stop=(i == 9),
                                 skip_group_check=True)
            warm_sink = persist.tile([1, 8], F32)
            nc.vector.tensor_copy(warm_sink[:, :], warm_ps[0:1, 0:8])

            qT_sb = [persist.tile([128, nt], BF, tag=f"qT{p}", name=f"qT{p}")
                     for p in range(2)]
            kT_sb = [persist.tile([128, nt], BF, tag=f"kT{p}", name=f"kT{p}")
                     for p in range(2)]
            oT_sb = [persist.tile([128, nt], BF, tag=f"oT{p}", name=f"oT{p}")
                     for p in range(2)]
            v_sb = persist.tile([128, n_kt, VW], BF)

            # ---------------- QKV chain emitters ----------------
            def qk_chain(w_sb, bcol, dst, p, t):
                ps = ps_qkv.tile([128, 512], F32, tag="qkv")
                for ci in range(8):
                    nc.tensor.matmul(
                        ps[:, :],
                        w_sb[:, ci, p * 128:(p + 1) * 128],
                        xT_sb[:, ci, t * 512:(t + 1) * 512],
                        start=(ci == 0), stop=(ci == 7))
                nc.vector.tensor_scalar_add(dst[:, t * 512:(t + 1) * 512],
                                            ps[:, :], bqk_sb[:, bcol:bcol + 1])

            def v_chain(tt):
                ps = ps_qkv.tile([128, 512], F32, tag="qkv")
                for ci in range(8):
                    nc.tensor.matmul(
                        ps[:, :VW],
                        xT_sb[:, ci, tt * 128:(tt + 1) * 128],
                        wv_sb[:, ci, :],
                        start=(ci == 0), stop=False)
                nc.tensor.matmul(
                    ps[:, :VW], ones_sb[:, 0:128], bv_sb[:, :],
                    start=False, stop=True)
                nc.vector.tensor_copy(v_sb[:, tt, :], ps[:, :VW])

            # bqk_sb columns: 0,1 = q bias pair 0/1; 2,3 = k bias pair 0/1
            def k_chain(p, t):
                qk_chain(wk_sb, 2 + p, kT_sb[p], p, t)

            def q_chain(p, t):
                qk_chain(wq_sb, 0 + p, qT_sb[p], p, t)

            # Preamble: just enough for the first attention group to start.
            k_chain(0, 0)
            q_chain(0, 0)
            for t in range(1, n_tc):
                k_chain(0, t)

            # Remaining producer chains, drip-fed between attention groups.
            # Block order runs all pair-0 query chunks first, so after the
            # preamble only the V chains (paced 2 per group by the first
            # block's own consumption) and a short K/Q chain queue remain —
            # the pair-1 chains have the whole pair-0 sweep to trickle in.
            thunks = []
            for k in range(n_ktg):
                thunks.append(lambda tt=2 * k: v_chain(tt))
                thunks.append(lambda tt=2 * k + 1: v_chain(tt))
            kq = []
            for t in range(1, n_tc):
                kq.append(lambda t=t: q_chain(0, t))
                kq.append(lambda t=t: k_chain(1, t - 1))
            kq.append(lambda: k_chain(1, n_tc - 1))
            for t in range(n_tc):
                kq.append(lambda t=t: q_chain(1, t))

            # ---------------- attention + projection ----------------
            def make_proj(qt, nh, use_act=False, pool=None):
                # use_act: at the kernel tail ACT is idle, so route the
                # PSUM->SBUF copy there and keep the DVE free for the
                # normalization multiplies
                def proj():
                    pps = (pool or ps_qkv).tile(
                        [128, 512], F32,
                        tag="s" if pool is not None else "qkv", name="pps")
                    for pp in range(2):
                        nc.tensor.matmul(
                            pps[:, :],
                            oT_sb[pp][:, qt * 128:(qt + 1) * 128],
                            wp_sb[:, pp, nh * 512:(nh + 1) * 512],
                            start=(pp == 0), stop=(pp == 1))
                    ost = stage.tile([128, 512], F32, tag="ost", name="ost")
                    if use_act:
                        nc.scalar.copy(ost[:, :], pps[:, :])
                    else:
                        nc.vector.tensor_copy(ost[:, :], pps[:, :])
                    nc.sync.dma_start(
                        out[qt * 128:(qt + 1) * 128, nh * 512:(nh + 1) * 512],
                        ost[:, :])
                return proj



            def make_norm_rest(o_ps, sumsb, p, qc, last_block):
                # PE+DVE part of softmax normalization; emitted in the NEXT
                # block once sumsb is certainly ready, so no PE instruction
                # ever waits on the DVE copy and stalls the score stream.
                def norm_rest():
                    bc_ps = ps_qkv.tile([128, 512], F32, tag="qkv",
                                        name="bc_ps")
                    for hh in range(2):
                        nc.tensor.matmul(bc_ps[hh * 64:(hh + 1) * 64, :],
                                         ones_sb[0:1, 0:64],
                                         sumsb[0:1, hh * 512:(hh + 1) * 512],
                                         start=True, stop=True,
                                         skip_group_check=True)
                    bc_sb = bcbuf.tile([128, 512], F32, tag="bc",
                                       name="bc_sb")
                    nc.vector.reciprocal(bc_sb[:, :], bc_ps[:, :])
                    for hh in range(2):
                        nc.vector.tensor_mul(
                            oT_sb[p][hh * 64:(hh + 1) * 64,
                                     qc * 512:(qc + 1) * 512],
                            o_ps[0:64, hh * 512:(hh + 1) * 512],
                            bc_sb[hh * 64:(hh + 1) * 64, :])
                    # queue this chunk's projection now that oT is written
                    if p == 1 and not last_block:
                        for qt4 in range(4):
                            for nh in range(2):
                                deferred.append(make_proj(qc * 4 + qt4, nh))
                return norm_rest

            AVLAG = min(3, n_ktg - 1)
            deferred = []
            blocks = [(qc, 0) for qc in range(n_tc)] + \
                     [(qc, 1) for qc in range(n_tc)]
            npop = 1 if n_tc >= 4 else 2
            prev_norm = None
            for bi, (qc, p) in enumerate(blocks):
                    o_ps = ps_o.tile([128, 1024], F32, tag="o", name="o_ps")
                    first_block = (bi == 0)
                    avq = []
                    for ktg in range(n_ktg):
                        if first_block:
                            for _ in range(2):
                                if thunks:
                                    thunks.pop(0)()
                        else:
                            # K/Q chains may feed this very group's S matmuls,
                            # so they must be emitted before them
                            for _ in range(npop):
                                if kq:
                                    kq.pop(0)()
                        s_ps = [ps_s.tile([128, 1024], F32, tag="s",
                                             name=f"s_ps{_h}")
                                for _h in range(2)]
                        for j in range(2):
                            kt = ktg * 2 + j
                            for hh in range(2):
                                nc.tensor.matmul(
                                    s_ps[hh][:, j * 512:(j + 1) * 512],
                                    kT_sb[p][hh * 64:(hh + 1) * 64,
                                             kt * 128:(kt + 1) * 128],
                                    qT_sb[p][hh * 64:(hh + 1) * 64,
                                             qc * 512:(qc + 1) * 512],
                                    start=True, stop=True)
                        pt = [pt_pool.tile([128, 1024], BF, tag="pt",
                                           name=f"pt{_h}")
                              for _h in range(2)]
                        for hh in range(2):
                            nc.scalar.activation(pt[hh][:, :], s_ps[hh][:, :],
                                                 AF.Exp, scale=SCALE)
                        if ktg == min(2, n_ktg - 1) and prev_norm is not None:
                            prev_norm()
                            prev_norm = None
                        else:
                            for _ in range(2 if len(deferred) > 1 else 1):
                                if deferred:
                                    deferred.pop(0)()

                        def make_av(ktg, pt):
                            def av():
                                for hh in range(2):
                                    h = 2 * p + hh
                                    for j in range(2):
                                        kt = ktg * 2 + j
                                        nc.tensor.matmul(
                                            o_ps[0:65,
                                                 hh * 512:(hh + 1) * 512],
                                            v_sb[:, kt, h * 65:(h + 1) * 65],
                                            pt[hh][:, j * 512:(j + 1) * 512],
                                            start=(ktg == 0 and j == 0),
                                            stop=(ktg == n_ktg - 1 and j == 1),
                                            skip_group_check=True)
                            return av

                        avq.append(make_av(ktg, pt))
                        if len(avq) > AVLAG:
                            avq.pop(0)()
                    while avq:
                        avq.pop(0)()
                    # start of normalization: pull the denominator row out of
                    # PSUM right after the last AV lands
                    sumsb = small.tile([1, 1024], BF, tag="sums")
                    nc.vector.tensor_copy(sumsb[:, :], o_ps[64:65, :])
                    if bi == len(blocks) - 1:
                        tail_state = (o_ps, sumsb, qc)
                        prev_norm = None
                    else:
                        prev_norm = make_norm_rest(
                            o_ps, sumsb, p, qc, last_block=False)
            # tail: last block's normalization, split per q-tile so each
            # projection starts as soon as its oT columns are normalized
            lb_o_ps, lb_sumsb, lb_qc = tail_state
            bc_ps = ps_qkv.tile([128, 512], F32, tag="qkv", name="bc_ps_t")
            for hh in range(2):
                nc.tensor.matmul(bc_ps[hh * 64:(hh + 1) * 64, :],
                                 ones_sb[0:1, 0:64],
                                 lb_sumsb[0:1, hh * 512:(hh + 1) * 512],
                                 start=True, stop=True,
                                 skip_group_check=True)
            bc_sb = bcbuf.tile([128, 512], F32, tag="bc", name="bc_sb_t")
            nc.vector.reciprocal(bc_sb[:, :], bc_ps[:, :])
            while deferred:
                deferred.pop(0)()
            for qt4 in range(4):
                qt = lb_qc * 4 + qt4
                for hh in range(2):
                    nc.vector.tensor_mul(
                        oT_sb[1][hh * 64:(hh + 1) * 64,
                                 qt * 128:(qt + 1) * 128],
                        lb_o_ps[0:64, hh * 512 + qt4 * 128:
                                hh * 512 + (qt4 + 1) * 128],
                        bc_sb[hh * 64:(hh + 1) * 64,
                              qt4 * 128:(qt4 + 1) * 128])
                for nh in range(2):
                    make_proj(qt, nh, use_act=True,
                              pool=ps_s if nh == 1 else None)()
                while deferred:
                    deferred.pop(0)()
            while deferred:
                deferred.pop(0)()
            assert not thunks and not kq, "producer chains never emitted"

    nc.finalize()
    return nc


def make_core_inputs(x, W_qkv, b_qkv, W_proj, nt=NT):
    """Host-side shard prep: returns in_maps list for the 8 cores."""
    in_maps = []
    for core in range(NCORES):
        b, g = divmod(core, NCORES // B)
        lo, hi = g * DQ, (g + 1) * DQ
        xTb = np.ascontiguousarray(x[b].T).astype(BF16)
        wq_c = np.ascontiguousarray(W_qkv[:, lo:hi]).astype(BF16)
        wk_c = np.ascontiguousarray(W_qkv[:, C + lo:C + hi]).astype(BF16)
        wv_full = W_qkv[:, 2 * C + lo:2 * C + hi]
        wv_c = np.zeros((C, VW), dtype=BF16)
        bv_c = np.zeros((1, VW), dtype=BF16)
        for h in range(HPC):
            wv_c[:, h * 65:h * 65 + 64] = wv_full[:, h * 64:(h + 1) * 64].astype(BF16)
            bv_c[0, h * 65:h * 65 + 64] = b_qkv[2 * C + lo + h * 64:
                                                2 * C + lo + (h + 1) * 64].astype(BF16)
            bv_c[0, h * 65 + 64] = 1.0
        wp_c = np.ascontiguousarray(W_proj[lo:hi, :]).astype(BF16)
        bqk_c = np.stack([
            b_qkv[lo:lo + 128], b_qkv[lo + 128:hi],
            b_qkv[C + lo:C + lo + 128], b_qkv[C + lo + 128:C + hi],
        ], axis=1).astype(np.float32)
        in_maps.append({
            "xT": xTb[:, :nt].copy(), "wq": wq_c, "wk": wk_c, "wv": wv_c,
            "wp": wp_c, "bqk": bqk_c, "bv": bv_c,
        })
    return in_maps


_prog_cache = {}


def _get_program(nt=NT):
    if nt not in _prog_cache:
        _prog_cache[nt] = build_program(nt)
    return _prog_cache[nt]


def kernel(x, W_qkv, b_qkv, W_proj, b_proj, _run_kwargs=None):
    x = np.asarray(x, dtype=np.float32)
    W_qkv = np.asarray(W_qkv, dtype=np.float32)
    b_qkv = np.asarray(b_qkv, dtype=np.float32)
    W_proj = np.asarray(W_proj, dtype=np.float32)
    b_proj = np.asarray(b_proj, dtype=np.float32)

    nc = _get_program()
    in_maps = make_core_inputs(x, W_qkv, b_qkv, W_proj)
    res = run_bass_kernel_spmd(nc, in_maps, core_ids=list(range(NCORES)),
                               **(_run_kwargs or {}))
    out = np.zeros((B, NT, C), dtype=np.float32)
    for core in range(NCORES):
        b = core // (NCORES // B)
        out[b] += res.results[core]["out_p"]
    out += b_proj[None, None, :]
    if _run_kwargs:
        kernel.last_results = res
    return out


# revision 44
# speedup vs baseline: 1.0125x; 1.0076x over previous
"""Trainium2 Bass kernel for multi-head self-attention (B=2, N=2048, C=1024, H=16, d=64).

Sharding: 8 cores = 2 batches x 4 head-groups (4 heads each). Each core computes
QKV for its heads (column-sliced W_qkv), full attention over its heads, and a
row-sliced partial of the output projection. Host sums the 4 partials per batch
and adds b_proj.

Device dataflow (per core, all matmuls bf16 with fp32 PSUM accumulation):
  - x^T is loaded [C, N] so Q^T/K^T come out as [head*d, N] (d on partitions),
    which is exactly the lhsT/rhs layout the scores matmul wants.
  - S^T tile [128 keys, 512 queries] = (K^T chunk)^T-matmul(Q^T chunk), K=64
    contraction; the two heads of a pair sit at partition offsets 0/64 so their
    matmuls occupy disjoint PE row-groups and run concurrently.
  - softmax skips the max-subtraction (scores are ~N(0,1); exp is safe in fp32)
    so exp(scale*S) is a single ACT pass straight out of PSUM, cast to bf16.
  - V carries an appended ones column (65th), so the attention-output matmul
    accumulates both O^T rows (0..63) and the softmax denominators (row 64).
  - normalization: denominator row broadcast to 64 partitions via K=1
    ones-matmuls, one 128-lane reciprocal, then vector multiplies into O^T
    bf16. Its PE part is emitted two groups into the NEXT block so no PE
    instruction ever waits on a DVE copy (in-order queues).
  - projection: out[q,c] = sum_p O^T-pair-chunk^T @ W_proj rows, fp32 out via
    DMA; emission deferred into later groups to keep ACT fed.
  - scheduling: AV matmuls lag S/exp by 4 groups (software pipeline), producer
    chains drip-feed between attention groups, block order runs all pair-0
    chunks before pair-1, and the PE is HAM-warmed during the initial DMAs.
"""

import sys

sys.path.insert(0, "/opt/trn_rl_repo")

import numpy as np
import ml_dtypes

import concourse.bass as bass
import concourse.tile as tile
from concourse import bacc, mybir
from concourse.bass_utils import run_bass_kernel_spmd

BF16 = ml_dtypes.bfloat16
F32 = mybir.dt.float32
BF = mybir.dt.bfloat16
AF = mybir.ActivationFunctionType

B, NT, C, H, D = 2, 2048, 1024, 16, 64
NCORES = 8
HPC = 4  # heads per core
DQ = HPC * D  # 256 c_out per q/k/v slice
VW = HPC * (D + 1)  # 260: V with a ones column per head
SCALE = D ** -0.5


def build_program(nt=NT):
    """Build the SPMD Bass program. nt parametrized so a small version can be
    simulated quickly in CoreSim."""
    n_tc = nt // 512  # 512-token chunks
    n_kt = nt // 128  # 128-key tiles
    n_ktg = nt // 256  # groups of 2 key tiles (one exp per 1024 cols)

    nc = bacc.Bacc("TRN2", target_bir_lowering=False, debug=False,
                   num_devices=NCORES)

    xT = nc.dram_tensor("xT", [C, nt], BF, kind="ExternalInput").ap()
    wq = nc.dram_tensor("wq", [C, DQ], BF, kind="ExternalInput").ap()
    wk = nc.dram_tensor("wk", [C, DQ], BF, kind="ExternalInput").ap()
    wv = nc.dram_tensor("wv", [C, VW], BF, kind="ExternalInput").ap()
    wp = nc.dram_tensor("wp", [DQ, C], BF, kind="ExternalInput").ap()
    bqk = nc.dram_tensor("bqk", [128, 4], F32, kind="ExternalInput").ap()
    bv = nc.dram_tensor("bv", [1, VW], BF, kind="ExternalInput").ap()
    out = nc.dram_tensor("out_p", [nt, C], F32, kind="ExternalOutput").ap()

    with tile.TileContext(nc) as tc:
        with (
            tc.tile_pool(name="persist", bufs=1) as persist,
            tc.tile_pool(name="pt_pool", bufs=8) as pt_pool,
            tc.tile_pool(name="stage", bufs=4) as stage,
            tc.tile_pool(name="bcbuf", bufs=3) as bcbuf,
            tc.tile_pool(name="small", bufs=6) as small,
            tc.tile_pool(name="ps_qkv", bufs=2, space="PSUM") as ps_qkv,
            tc.tile_pool(name="ps_s", bufs=2, space="PSUM") as ps_s,
            tc.tile_pool(name="ps_o", bufs=1, space="PSUM") as ps_o,
        ):
            # ---------------- persistent SBUF state ----------------
            # load order matters: wk + xT feed the first K^T chains; wv/wp
            # are only needed once attention is underway.
            xT_sb = persist.tile([128, 8, nt], BF)
            wq_sb = persist.tile([128, 8, DQ], BF)
            wk_sb = persist.tile([128, 8, DQ], BF)
            wv_sb = persist.tile([128, 8, VW], BF)
            bqk_sb = persist.tile([128, 4], F32)
            bv_sb = persist.tile([1, VW], BF)
            wp_sb = persist.tile([128, 2, C], BF)
            # Few big DMA instructions (the ~1.3us sequencer issue cost per
            # DMA dominates; transfers run on 16 parallel DMA engines).
            # x^T rides the SP hardware queue in 512-token chunks so the first
            # K/Q chains start early; weights ride the idle Pool (SWDGE) queue.
            xT3 = xT.rearrange("(po pi) n -> pi po n", pi=128)
            wk3 = wk.rearrange("(po pi) c -> pi po c", pi=128)
            wq3 = wq.rearrange("(po pi) c -> pi po c", pi=128)
            wv3 = wv.rearrange("(po pi) c -> pi po c", pi=128)
            wp3 = wp.rearrange("(po pi) c -> pi po c", pi=128)
            def _xt(t):
                if t == 0:
                    return
                sl = slice(t * 512, (t + 1) * 512)
                nc.sync.dma_start(xT_sb[:, 0:4, sl], xT3[:, 0:4, sl])
                nc.sync.dma_start(xT_sb[:, 4:8, sl], xT3[:, 4:8, sl])

            nc.sync.dma_start(bqk_sb[:], bqk)
            nc.sync.dma_start(wk_sb[:], wk3)
            # first token chunk in two halves so the first K-chain's matmuls
            # start as soon as contraction-chunks 0..3 land
            nc.sync.dma_start(xT_sb[:, 0:4, 0:512], xT3[:, 0:4, 0:512])
            nc.sync.dma_start(xT_sb[:, 4:8, 0:512], xT3[:, 4:8, 0:512])
            nc.sync.dma_start(wq_sb[:], wq3)
            for t in range(1, n_tc):
                _xt(t)
                if t == 1:
                    nc.sync.dma_start(bv_sb[:], bv)
                    nc.sync.dma_start(wv_sb[:], wv3)
                elif t == 2:
                    nc.sync.dma_start(wp_sb[:], wp3)
            if n_tc < 3:
                nc.sync.dma_start(bv_sb[:], bv)
                nc.sync.dma_start(wv_sb[:], wv3)
                nc.sync.dma_start(wp_sb[:], wp3)
            ones_sb = persist.tile([1, 512], BF)
            nc.vector.memset(ones_sb[:], 1.0)
            # warm the PE clock (HAM) with throwaway matmuls while the first
            # DMAs land, so the first real chains run at full rate
            warm_ps = ps_qkv.tile([128, 512], F32, tag="qkv", name="warm_ps")
            for i in range(10):
                nc.tensor.matmul(warm_ps[:, :], ones_sb[:, 0:128],
                                 ones_sb[:, :], start=(i == 0), stop=(i == 9),
                                 skip_group_check=True)
            warm_sink = persist.tile([1, 8], F32)
            nc.vector.tensor_copy(warm_sink[:, :], warm_ps[0:1, 0:8])

            qT_sb = [persist.tile([128, nt], BF, tag=f"qT{p}", name=f"qT{p}")
                     for p in range(2)]
            kT_sb = [persist.tile([128, nt], BF, tag=f"kT{p}", name=f"kT{p}")
                     for p in range(2)]
            oT_sb = [persist.tile([128, nt], BF, tag=f"oT{p}", name=f"oT{p}")
                     for p in range(2)]
            v_sb = persist.tile([128, n_kt, VW], BF)

            # ---------------- QKV chain emitters ----------------
            def qk_chain(w_sb, bcol, dst, p, t):
                ps = ps_qkv.tile([128, 512], F32, tag="qkv")
                for ci in range(8):
                    nc.tensor.matmul(
                        ps[:, :],
                        w_sb[:, ci, p * 128:(p + 1) * 128],
                        xT_sb[:, ci, t * 512:(t + 1) * 512],
                        start=(ci == 0), stop=(ci == 7))
                nc.vector.tensor_scalar_add(dst[:, t * 512:(t + 1) * 512],
                                            ps[:, :], bqk_sb[:, bcol:bcol + 1])

            def v_chain(tt):
                ps = ps_qkv.tile([128, 512], F32, tag="qkv")
                for ci in range(8):
                    nc.tensor.matmul(
                        ps[:, :VW],
                        xT_sb[:, ci, tt * 128:(tt + 1) * 128],
                        wv_sb[:, ci, :],
                        start=(ci == 0), stop=False)
                nc.tensor.matmul(
                    ps[:, :VW], ones_sb[:, 0:128], bv_sb[:, :],
                    start=False, stop=True)
                nc.vector.tensor_copy(v_sb[:, tt, :], ps[:, :VW])

            # bqk_sb columns: 0,1 = q bias pair 0/1; 2,3 = k bias pair 0/1
            def k_chain(p, t):
                qk_chain(wk_sb, 2 + p, kT_sb[p], p, t)

            def q_chain(p, t):
                qk_chain(wq_sb, 0 + p, qT_sb[p], p, t)

            # Preamble: just enough for the first attention group to start.
            k_chain(0, 0)
            q_chain(0, 0)
            for t in range(1, n_tc):
                k_chain(0, t)

            # Remaining producer chains, drip-fed between attention groups.
            # Block order runs all pair-0 query chunks first, so after the
            # preamble only the V chains (paced 2 per group by the first
            # block's own consumption) and a short K/Q chain queue remain —
            # the pair-1 chains have the whole pair-0 sweep to trickle in.
            thunks = []
            for k in range(n_ktg):
                thunks.append(lambda tt=2 * k: v_chain(tt))
                thunks.append(lambda tt=2 * k + 1: v_chain(tt))
            kq = []
            for t in range(1, n_tc):
                kq.append(lambda t=t: q_chain(0, t))
                kq.append(lambda t=t: k_chain(1, t - 1))
            kq.append(lambda: k_chain(1, n_tc - 1))
            for t in range(n_tc):
                kq.append(lambda t=t: q_chain(1, t))

            # ---------------- attention + projection ----------------
            def make_proj(qt, nh, use_act=False, pool=None):
                # use_act: at the kernel tail ACT is idle, so route the
                # PSUM->SBUF copy there and keep the DVE free for the
                # normalization multiplies
                def proj():
                    pps = (pool or ps_qkv).tile(
                        [128, 512], F32,
                        tag="s" if pool is not None else "qkv", name="pps")
                    for pp in range(2):
                        nc.tensor.matmul(
                            pps[:, :],
                            oT_sb[pp][:, qt * 128:(qt + 1) * 128],
                            wp_sb[:, pp, nh * 512:(nh + 1) * 512],
                            start=(pp == 0), stop=(pp == 1))
                    ost = stage.tile([128, 512], F32, tag="ost", name="ost")
                    if use_act:
                        nc.scalar.copy(ost[:, :], pps[:, :])
                    else:
                        nc.vector.tensor_copy(ost[:, :], pps[:, :])
                    nc.sync.dma_start(
                        out[qt * 128:(qt + 1) * 128, nh * 512:(nh + 1) * 512],
                        ost[:, :])
                return proj



            def make_norm_rest(o_ps, sumsb, p, qc, last_block):
                # PE+DVE part of softmax normalization; emitted in the NEXT
                # block once sumsb is certainly ready, so no PE instruction
                # ever waits on the DVE copy and stalls the score stream.
                def norm_rest():
                    bc_ps = ps_qkv.tile([128, 512], F32, tag="qkv",
                                        name="bc_ps")
                    for hh in range(2):
                        nc.tensor.matmul(bc_ps[hh * 64:(hh + 1) * 64, :],
                                         ones_sb[0:1, 0:64],
                                         sumsb[0:1, hh * 512:(hh + 1) * 512],
                                         start=True, stop=True,
                                         skip_group_check=True)
                    bc_sb = bcbuf.tile([128, 512], F32, tag="bc",
                                       name="bc_sb")
                    nc.vector.reciprocal(bc_sb[:, :], bc_ps[:, :])
                    for hh in range(2):
                        nc.vector.tensor_mul(
                            oT_sb[p][hh * 64:(hh + 1) * 64,
                                     qc * 512:(qc + 1) * 512],
                            o_ps[0:64, hh * 512:(hh + 1) * 512],
                            bc_sb[hh * 64:(hh + 1) * 64, :])
                    # queue this chunk's projection now that oT is written
                    if p == 1 and not last_block:
                        for qt4 in range(4):
                            for nh in range(2):
                                deferred.append(make_proj(qc * 4 + qt4, nh))
                return norm_rest

            AVLAG = min(3, n_ktg - 1)
            deferred = []
            blocks = [(qc, 0) for qc in range(n_tc)] + \
                     [(qc, 1) for qc in range(n_tc)]
            npop = 1 if n_tc >= 4 else 2
            prev_norm = None
            for bi, (qc, p) in enumerate(blocks):
                    o_ps = ps_o.tile([128, 1024], F32, tag="o", name="o_ps")
                    first_block = (bi == 0)
                    avq = []
                    for ktg in range(n_ktg):
                        if first_block:
                            for _ in range(2):
                                if thunks:
                                    thunks.pop(0)()
                        else:
                            # K/Q chains may feed this very group's S matmuls,
                            # so they must be emitted before them
                            for _ in range(npop):
                                if kq:
                                    kq.pop(0)()
                        s_ps = [ps_s.tile([128, 1024], F32, tag="s",
                                             name=f"s_ps{_h}")
                                for _h in range(2)]
                        for j in range(2):
                            kt = ktg * 2 + j
                            for hh in range(2):
                                nc.tensor.matmul(
                                    s_ps[hh][:, j * 512:(j + 1) * 512],
                                    kT_sb[p][hh * 64:(hh + 1) * 64,
                                             kt * 128:(kt + 1) * 128],
                                    qT_sb[p][hh * 64:(hh + 1) * 64,
                                             qc * 512:(qc + 1) * 512],
                                    start=True, stop=True)
                        pt = [pt_pool.tile([128, 1024], BF, tag="pt",
                                           name=f"pt{_h}")
                              for _h in range(2)]
                        for hh in range(2):
                            nc.scalar.activation(pt[hh][:, :], s_ps[hh][:, :],
                                                 AF.Exp, scale=SCALE)
                        if ktg == min(2, n_ktg - 1) and prev_norm is not None:
                            prev_norm()
                            prev_norm = None
                        else:
                            for _ in range(2 if len(deferred) > 1 else 1):
                                if deferred:
                                    deferred.pop(0)()

                        def make_av(ktg, pt):
                            def av():
                                for hh in range(2):
                                    h = 2 * p + hh
                                    for j in range(2):
                                        kt = ktg * 2 + j
                                        nc.tensor.matmul(
                                            o_ps[0:65,
                                                 hh * 512:(hh + 1) * 512],
                                            v_sb[:, kt, h * 65:(h + 1) * 65],
                                            pt[hh][:, j * 512:(j + 1) * 512],
                                            start=(ktg == 0 and j == 0),
                                            stop=(ktg == n_ktg - 1 and j == 1),
                                            skip_group_check=True)
                            return av

                        avq.append(make_av(ktg, pt))
                        if len(avq) > AVLAG:
                            avq.pop(0)()
                    while avq:
                        avq.pop(0)()
                    # start of normalization: pull the denominator row out of
                    # PSUM right after the last AV lands
                    sumsb = small.tile([1, 1024], BF, tag="sums")
                    nc.vector.tensor_copy(sumsb[:, :], o_ps[64:65, :])
                    if bi == len(blocks) - 1:
                        tail_state = (o_ps, sumsb, qc)
                        prev_norm = None
                    else:
                        prev_norm = make_norm_rest(
                            o_ps, sumsb, p, qc, last_block=False)
            # tail: last block's normalization, split per q-tile so each
            # projection starts as soon as its oT columns are normalized
            lb_o_ps, lb_sumsb, lb_qc = tail_state
            bc_ps = ps_qkv.tile([128, 512], F32, tag="qkv", name="bc_ps_t")
            for hh in range(2):
                nc.tensor.matmul(bc_ps[hh * 64:(hh + 1) * 64, :],
                                 ones_sb[0:1, 0:64],
                                 lb_sumsb[0:1, hh * 512:(hh + 1) * 512],
                                 start=True, stop=True,
                                 skip_group_check=True)
            bc_sb = bcbuf.tile([128, 512], F32, tag="bc", name="bc_sb_t")
            nc.vector.reciprocal(bc_sb[:, :], bc_ps[:, :])
            while deferred:
                deferred.pop(0)()
            for qt4 in range(4):
                qt = lb_qc * 4 + qt4
                for hh in range(2):
                    nc.vector.tensor_mul(
                        oT_sb[1][hh * 64:(hh + 1) * 64,
                                 qt * 128:(qt + 1) * 128],
                        lb_o_ps[0:64, hh * 512 + qt4 * 128:
                                hh * 512 + (qt4 + 1) * 128],
                        bc_sb[hh * 64:(hh + 1) * 64,
                              qt4 * 128:(qt4 + 1) * 128])
                for nh in range(2):
                    make_proj(qt, nh, use_act=True,
                              pool=ps_s if nh == 1 else None)()
                while deferred:
                    deferred.pop(0)()
            while deferred:
                deferred.pop(0)()
            assert not thunks and not kq, "producer chains never emitted"

    nc.finalize()
    return nc


def make_core_inputs(x, W_qkv, b_qkv, W_proj, nt=NT):
    """Host-side shard prep: returns in_maps list for the 8 cores."""
    in_maps = []
    for core in range(NCORES):
        b, g = divmod(core, NCORES // B)
        lo, hi = g * DQ, (g + 1) * DQ
        xTb = np.ascontiguousarray(x[b].T).astype(BF16)
        wq_c = np.ascontiguousarray(W_qkv[:, lo:hi]).astype(BF16)
        wk_c = np.ascontiguousarray(W_qkv[:, C + lo:C + hi]).astype(BF16)
        wv_full = W_qkv[:, 2 * C + lo:2 * C + hi]
        wv_c = np.zeros((C, VW), dtype=BF16)
        bv_c = np.zeros((1, VW), dtype=BF16)
        for h in range(HPC):
            wv_c[:, h * 65:h * 65 + 64] = wv_full[:, h * 64:(h + 1) * 64].astype(BF16)
            bv_c[0, h * 65:h * 65 + 64] = b_qkv[2 * C + lo + h * 64:
                                                2 * C + lo + (h + 1) * 64].astype(BF16)
            bv_c[0, h * 65 + 64] = 1.0
        wp_c = np.ascontiguousarray(W_proj[lo:hi, :]).astype(BF16)
        bqk_c = np.stack([
            b_qkv[lo:lo + 128], b_qkv[lo + 128:hi],
            b_qkv[C + lo:C + lo + 128], b_qkv[C + lo + 128:C + hi],
        ], axis=1).astype(np.float32)
        in_maps.append({
            "xT": xTb[:, :nt].copy(), "wq": wq_c, "wk": wk_c, "wv": wv_c,
            "wp": wp_c, "bqk": bqk_c, "bv": bv_c,
        })
    return in_maps


_prog_cache = {}


def _get_program(nt=NT):
    if nt not in _prog_cache:
        _prog_cache[nt] = build_program(nt)
    return _prog_cache[nt]


def kernel(x, W_qkv, b_qkv, W_proj, b_proj, _run_kwargs=None):
    x = np.asarray(x, dtype=np.float32)
    W_qkv = np.asarray(W_qkv, dtype=np.float32)
    b_qkv = np.asarray(b_qkv, dtype=np.float32)
    W_proj = np.asarray(W_proj, dtype=np.float32)
    b_proj = np.asarray(b_proj, dtype=np.float32)

    nc = _get_program()
    in_maps = make_core_inputs(x, W_qkv, b_qkv, W_proj)
    res = run_bass_kernel_spmd(nc, in_maps, core_ids=list(range(NCORES)),
                               **(_run_kwargs or {}))
    out = np.zeros((B, NT, C), dtype=np.float32)
    for core in range(NCORES):
        b = core // (NCORES // B)
        out[b] += res.results[core]["out_p"]
    out += b_proj[None, None, :]
    if _run_kwargs:
        kernel.last_results = res
    return out
